# revision 13
# baseline (speedup 1.0000x reference)
"""GAT layer (LayerNorm -> GATConv(heads=1) -> residual ReLU) on 8 trn2 NeuronCores.

Sharding: destination-node parallel. Each core owns N/8 nodes: it computes the
node transform for its shard, shards are AllGathered (bf16, 256 B rows), and
each core processes the edges whose destination falls in its shard.

Per-edge source records are fetched with 256 B non-transposed dma_gathers.
SWDGE descriptor generation (~7.8 ns/row, Q7-bound) is the critical resource:
  * slots are packed at (group, segment) granularity -- columns may straddle
    two dst blocks; boundary columns simply get one extra accumulating
    matmul per extra block (one-hot tables are emitted per (column, block)).
  * own-shard edges (src in the core's own shard) gather from the local
    table and are issued before the AllGather, hiding their descgen.
  * the Vector engine avoids 2-port DVE modes during phase B (they lock
    GpSimd out of SBUF): one-hot operands are host-precomputed and streamed,
    per-edge ee scaling runs on the Scalar engine.

The gathered row packs a_src INTO the feature vector: row[jmax] =
sum_f att_src[f]*xp[f] with jmax = argmax|att_src|; the aggregated feature
jmax is recovered post-scatter from the same linear identity. a_dst is looked
up on the tensor engine (ohT[r,e] one-hot times the block's a_dst column).
ee = exp(leakyrelu(a_src + a_dst)); the scatter-add is a one-hot matmul with
an ee column in the rhs producing the softmax denominator.
"""

import ml_dtypes
import numpy as np

import concourse.bacc as bacc
import concourse.mybir as mybir
import concourse.tile as tile
from concourse.bass_utils import run_bass_kernel_spmd

F32 = mybir.dt.float32
BF16 = mybir.dt.bfloat16
I16 = mybir.dt.int16
I32 = mybir.dt.int32
AX = mybir.AxisListType
OP = mybir.AluOpType
AF = mybir.ActivationFunctionType
NPBF = ml_dtypes.bfloat16
NPF8 = ml_dtypes.float8_e4m3
FP8 = mybir.dt.float8e4

N = 50000
D = 128
E = 600000
NCORES = 8
SHARD = N // NCORES            # 6250
NBLK = (SHARD + 127) // 128    # 49 dst blocks per core
PAD_SHARD = NBLK * 128         # 6272
LAST_ROWS = SHARD - (NBLK - 1) * 128  # 106
HALF = 32768
NEG_SLOPE = 0.2
LN_EPS = 1e-5
GBLK = 4                       # dst blocks per gather group
ABLK = 4                       # blocks per phase-A giga-iteration


def _build_program(layout, jmax, inv_ajmax):
    """layout: static per-group description (shared by all cores):
    list of dicts with keys
      blocks:   [b...]
      ncols:    {s: cols for seg s}            (s in 0=own,1=lo,2=hi)
      col0:     {s: global fidx column of seg s's first column}
      entries:  {s: [(b, j, eidx), ...]}       j = col within seg,
                                               eidx = global oh-entry index
      e0own / e0lohi: global entry range starts for the group's own / lo+hi
                      entry blocks (own entries contiguous; lo+hi contiguous)
      nent_own / nent_lohi
    plus layout_tot = dict(ncols_tot, nent_tot).
    """
    groups, tot = layout
    CBG_MAX = max(g["ncols"][1] + g["ncols"][2] for g in groups)
    OWN_MAX = max(g["ncols"][0] for g in groups)
    ENT_MAX = max(g["nent_own"] + g["nent_lohi"] for g in groups)
    CB = tot["ncols_tot"]
    NENT = tot["nent_tot"]

    nc = bacc.Bacc("TRN2", num_devices=NCORES, debug=False)

    x_shard = nc.dram_tensor("x_shard", [PAD_SHARD, D], F32, kind="ExternalInput")
    wext = nc.dram_tensor("wext", [D, 129], BF16, kind="ExternalInput")
    c2b = nc.dram_tensor("c2b", [128, 129], F32, kind="ExternalInput")
    ident = nc.dram_tensor("ident", [128, 128], F32, kind="ExternalInput")
    attb = nc.dram_tensor("attb", [128, 128], F32, kind="ExternalInput")
    feat_idx = nc.dram_tensor("feat_idx", [128, CB * 8], I16, kind="ExternalInput")
    oh_t = nc.dram_tensor("oh_t", [128, NENT * 128], FP8, kind="ExternalInput")
    ohT_t = nc.dram_tensor("ohT_t", [128, NENT * 128], FP8, kind="ExternalInput")
    out_shard = nc.dram_tensor("out_shard", [SHARD, D], F32, kind="ExternalOutput")

    with tile.TileContext(nc) as tc:
        with (
            tc.tile_pool(name="dram", bufs=1, space="DRAM") as dram,
            tc.tile_pool(name="consts", bufs=1) as cpool,
            tc.tile_pool(name="xres", bufs=1) as xpool,
            tc.tile_pool(name="ownt", bufs=1) as ownpool,
        ):
            xp_shard = dram.tile([SHARD, D], BF16)
            xp_full = dram.tile([N, D], BF16, addr_space="Shared")

            ident_sb = cpool.tile([128, 128], F32)
            nc.sync.dma_start(ident_sb[:], ident[:, :])
            attb_sb = cpool.tile([128, 128], F32)
            nc.sync.dma_start(attb_sb[:], attb[:, :])
            wext_sb = cpool.tile([D, 129], BF16)
            nc.sync.dma_start(wext_sb[:], wext[:, :])
            c2b_sb = cpool.tile([128, 129], F32)
            nc.sync.dma_start(c2b_sb[:], c2b[:, :])
            eps_sb = cpool.tile([128, 1], F32)
            nc.vector.memset(eps_sb[:], LN_EPS)
            fidx_sb = cpool.tile([128, CB * 8], I16)
            nc.sync.dma_start(fidx_sb[:], feat_idx[:, :])
            adst_sb = cpool.tile([128, NBLK], BF16)

            # x loaded 4 blocks per tile: partition p = row p of each block
            nga = (NBLK + ABLK - 1) // ABLK
            x_tiles = []
            for a in range(nga):
                nb = min(ABLK, NBLK - a * ABLK)
                xt = xpool.tile([128, ABLK, D], F32, tag=f"x4_{a}")
                nc.sync.dma_start(
                    xt[:, 0:nb, :],
                    x_shard[a * ABLK * 128 : (a * ABLK + nb) * 128, :].rearrange(
                        "(a p) b -> p a b", p=128
                    ),
                )
                x_tiles.append(xt)

            def xres(b):
                return x_tiles[b // ABLK][:, b % ABLK, :]

            # ---------------- Phase A: node transform on own shard ---------
            with (
                tc.tile_pool(name="a_small", bufs=6) as spool,
                tc.tile_pool(name="a_sq", bufs=3) as sqpool,
                tc.tile_pool(name="a_xnp", bufs=6) as xnppool,
                tc.tile_pool(name="a_xnpT", bufs=6) as xnptpool,
                tc.tile_pool(name="a_xpe", bufs=6) as xpepool,
                tc.tile_pool(name="a_ps_t", bufs=3, space="PSUM") as psa,
                tc.tile_pool(name="a_ps_m", bufs=3, space="PSUM") as psb,
            ):
                for a in range(nga):
                    nb = min(ABLK, NBLK - a * ABLK)
                    xt = x_tiles[a]
                    sum4 = spool.tile([128, ABLK], F32, tag="sum4")
                    nc.vector.tensor_reduce(sum4[:, 0:nb], xt[:, 0:nb, :], AX.X, OP.add)
                    sq4 = sqpool.tile([128, ABLK, D], F32)
                    nc.scalar.activation(sq4[:, 0:nb, :], xt[:, 0:nb, :], AF.Square)
                    ssq4 = spool.tile([128, ABLK], F32, tag="ssq4")
                    nc.vector.tensor_reduce(ssq4[:, 0:nb], sq4[:, 0:nb, :], AX.X, OP.add)
                    mu4 = spool.tile([128, ABLK], F32, tag="mu4")
                    nc.vector.tensor_scalar(mu4[:, 0:nb], sum4[:, 0:nb], 1.0 / D, None, OP.mult)
                    m24 = spool.tile([128, ABLK], F32, tag="m24")
                    nc.vector.tensor_tensor(m24[:, 0:nb], mu4[:, 0:nb], mu4[:, 0:nb], OP.mult)
                    var4 = spool.tile([128, ABLK], F32, tag="var4")
                    nc.vector.tensor_scalar(
                        var4[:, 0:nb], ssq4[:, 0:nb], 1.0 / D, None, OP.mult
                    )
                    nc.vector.tensor_tensor(
                        var4[:, 0:nb], var4[:, 0:nb], m24[:, 0:nb], OP.subtract
                    )
                    std4 = spool.tile([128, ABLK], F32, tag="std4")
                    nc.scalar.activation(std4[:, 0:nb], var4[:, 0:nb], AF.Sqrt, bias=eps_sb[:, 0:1])
                    rstd4 = spool.tile([128, ABLK], F32, tag="rstd4")
                    nc.vector.reciprocal(rstd4[:, 0:nb], std4[:, 0:nb])
                    for k in range(nb):
                        i = a * ABLK + k
                        rows = 128 if i < NBLK - 1 else LAST_ROWS
                        xnp = xnppool.tile([128, D], F32)
                        nc.vector.tensor_scalar(
                            xnp[:], xt[:, k, :], mu4[:, k : k + 1],
                            rstd4[:, k : k + 1], OP.subtract, OP.mult,
                        )
                        pt = psa.tile([128, 128], F32, space="PSUM")
                        nc.tensor.transpose(pt[:], xnp[:], ident_sb[:])
                        xnpT = xnptpool.tile([128, 128], BF16)
                        nc.scalar.copy(xnpT[:], pt[:])
                        pm = psb.tile([128, 129], F32, space="PSUM")
                        nc.tensor.matmul(
                            pm[:], lhsT=xnpT[:], rhs=wext_sb[:], start=True, stop=True
                        )
                        xpe = xpepool.tile([128, 129], BF16)
                        nc.vector.tensor_tensor(xpe[:], pm[:], c2b_sb[:], OP.add)
                        nc.sync.dma_start(
                            xp_shard[i * 128 : i * 128 + rows, :], xpe[:rows, 0:128]
                        )
                        nc.vector.tensor_copy(adst_sb[:, i : i + 1], xpe[:, 128:129])

            # AllGather triggers first (runs on the CC cores), then the
            # own-shard gathers' descgen overlaps the collective transfer
            nc.gpsimd.collective_compute(
                "AllGather",
                OP.bypass,
                replica_groups=[list(range(NCORES))],
                ins=[xp_shard[:, :]],
                outs=[xp_full[:, :]],
            )
            ownT = []
            for gi, g in enumerate(groups):
                ncol = g["ncols"][0]
                t = ownpool.tile([128, max(ncol, 1), D], BF16, tag=f"ownT{gi}")
                if ncol:
                    c0 = g["col0"][0]
                    nc.gpsimd.dma_gather(
                        out_ap=t[:, 0:ncol, :],
                        in_ap=xp_shard[:, :],
                        idxs_ap=fidx_sb[:, c0 * 8 : (c0 + ncol) * 8],
                        num_idxs=ncol * 128,
                        num_idxs_reg=ncol * 128,
                        elem_size=D,
                        single_packet=False,
                    )
                ownT.append(t)

            # ---------------- Phase B: edge aggregation --------------------
            with (
                tc.tile_pool(name="b_g", bufs=3) as gpool,
                tc.tile_pool(name="b_oh", bufs=2) as opool,
                tc.tile_pool(name="b_ohT", bufs=2) as otpool,
                tc.tile_pool(name="b_f", bufs=2) as fpool,
                tc.tile_pool(name="b_e", bufs=3) as epool,
                tc.tile_pool(name="b_o", bufs=4) as outpool,
                tc.tile_pool(name="b_pso", bufs=4, space="PSUM") as psopool,
                tc.tile_pool(name="b_psa", bufs=2, space="PSUM") as psapool,
            ):
                for gi, g in enumerate(groups):
                    blocks = g["blocks"]
                    nown = g["ncols"][0]
                    nlo, nhi = g["ncols"][1], g["ncols"][2]
                    cbl = nlo + nhi          # lo+hi cols in T
                    cbg = nown + cbl         # total cols this group
                    T = gpool.tile([128, CBG_MAX, D], BF16, tag="T")
                    if nlo:
                        c0 = g["col0"][1]
                        nc.gpsimd.dma_gather(
                            out_ap=T[:, 0:nlo, :],
                            in_ap=xp_full[0:HALF, :],
                            idxs_ap=fidx_sb[:, c0 * 8 : (c0 + nlo) * 8],
                            num_idxs=nlo * 128,
                            num_idxs_reg=nlo * 128,
                            elem_size=D,
                            single_packet=False,
                        )
                    if nhi:
                        c0 = g["col0"][2]
                        nc.gpsimd.dma_gather(
                            out_ap=T[:, nlo : nlo + nhi, :],
                            in_ap=xp_full[HALF:N, :],
                            idxs_ap=fidx_sb[:, c0 * 8 : (c0 + nhi) * 8],
                            num_idxs=nhi * 128,
                            num_idxs_reg=nhi * 128,
                            elem_size=D,
                            single_packet=False,
                        )

                    # oh/ohT entries: [own entries | lo+hi entries]
                    ne_own, ne_lohi = g["nent_own"], g["nent_lohi"]
                    nent = ne_own + ne_lohi
                    ohg = opool.tile([128, ENT_MAX * 128], FP8, tag="ohg")
                    ohTg = otpool.tile([128, ENT_MAX * 128], FP8, tag="ohTg")
                    if ne_own:
                        e0 = g["e0own"]
                        nc.sync.dma_start(
                            ohg[:, 0 : ne_own * 128],
                            oh_t[:, e0 * 128 : (e0 + ne_own) * 128],
                        )
                        nc.sync.dma_start(
                            ohTg[:, 0 : ne_own * 128],
                            ohT_t[:, e0 * 128 : (e0 + ne_own) * 128],
                        )
                    e0 = g["e0lohi"]
                    nc.sync.dma_start(
                        ohg[:, ne_own * 128 : nent * 128],
                        oh_t[:, e0 * 128 : (e0 + ne_lohi) * 128],
                    )
                    nc.sync.dma_start(
                        ohTg[:, ne_own * 128 : nent * 128],
                        ohT_t[:, e0 * 128 : (e0 + ne_lohi) * 128],
                    )

                    def tcol(j):
                        """(tile, local col) for group column j (own first)."""
                        if j < nown:
                            return ownT[gi], j
                        return T, j - nown

                    # gate: a_dst columns data-dependent on the gathers so the
                    # a_dst matmuls can't hoist into phase A
                    gn = len(blocks)
                    g0 = blocks[0]
                    tz = epool.tile([128, 1], F32, tag="tz")
                    src_gate = T if cbl else ownT[gi]
                    nc.vector.tensor_scalar(tz[:], src_gate[:, 0, 0:1], 0.0, None, OP.mult)
                    adst_g = epool.tile([128, GBLK], BF16, tag="adst_g")
                    nc.vector.tensor_scalar(
                        adst_g[:, 0:gn], adst_sb[:, g0 : g0 + gn], tz[:, 0:1],
                        None, OP.add,
                    )

                    # a_dst lookups: per (column, block) entry, accumulated
                    # into the column's ps_adst slot
                    ps_adst = psapool.tile([128, max(cbg, 1)], F32, space="PSUM")
                    ents = g["entries"]  # {s: [(b, j_seg, eidx_global)]}
                    # group-local column index: own seg cols [0, nown);
                    # lo cols [nown, nown+nlo); hi cols [nown+nlo, cbg)
                    colbase = {0: 0, 1: nown, 2: nown + nlo}

                    def eloc(s, el):
                        """global oh entry index -> column in ohg/ohTg tile"""
                        if s == 0:
                            return el - g["e0own"]
                        return el - g["e0lohi"] + ne_own

                    # first/last entry per column for start/stop flags
                    col_ents = {}
                    for s in (0, 1, 2):
                        for (b, j, el) in ents[s]:
                            col_ents.setdefault(colbase[s] + j, []).append(
                                (b, eloc(s, el))
                            )
                    for col in sorted(col_ents):
                        for idx, (b, el) in enumerate(col_ents[col]):
                            nc.tensor.matmul(
                                ps_adst[:, col : col + 1],
                                lhsT=ohTg[:, el * 128 : (el + 1) * 128],
                                rhs=adst_g[:, b - g0 : b - g0 + 1],
                                start=(idx == 0),
                                stop=(idx == len(col_ents[col]) - 1),
                            )

                    # ee = exp(leakyrelu(a_src + a_dst)) for the whole group
                    adst_bg = epool.tile([128, max(cbg, 1)], BF16, tag="adst_bg")
                    nc.vector.tensor_copy(adst_bg[:, 0:cbg], ps_adst[:, 0:cbg])
                    e1 = epool.tile([128, max(cbg, 1)], BF16, tag="e1")
                    if nown:
                        nc.vector.tensor_tensor(
                            e1[:, 0:nown], ownT[gi][:, 0:nown, jmax],
                            adst_bg[:, 0:nown], OP.add,
                        )
                    if cbl:
                        nc.vector.tensor_tensor(
                            e1[:, nown:cbg], T[:, 0:cbl, jmax],
                            adst_bg[:, nown:cbg], OP.add,
                        )
                    e2 = epool.tile([128, max(cbg, 1)], BF16, tag="e2")
                    nc.vector.tensor_scalar(
                        e2[:, 0:cbg], e1[:, 0:cbg], NEG_SLOPE, None, OP.mult
                    )
                    e3 = epool.tile([128, max(cbg, 1)], BF16, tag="e3")
                    nc.vector.tensor_tensor(
                        e3[:, 0:cbg], e2[:, 0:cbg], e1[:, 0:cbg], OP.max
                    )
                    ee = epool.tile([128, max(cbg, 1)], F32, tag="ee")
                    nc.scalar.activation(ee[:, 0:cbg], e3[:, 0:cbg], AF.Exp)

                    # rhs rows scaled by ee (Scalar engine), ee denom column
                    T2 = fpool.tile([128, CBG_MAX + OWN_MAX, D + 1], BF16, tag="T2")
                    for col in range(cbg):
                        tt, lj = tcol(col)
                        if col % 2 == 0:
                            nc.scalar.activation(
                                T2[:, col, 0:D], tt[:, lj, :], AF.Copy,
                                scale=ee[:, col : col + 1],
                            )
                        else:
                            nc.vector.tensor_scalar(
                                T2[:, col, 0:D], tt[:, lj, :],
                                ee[:, col : col + 1], None, OP.mult,
                            )
                    nc.vector.tensor_copy(T2[:, 0:cbg, D], ee[:, 0:cbg])

                    # per-block scatter: all entries of block b accumulate
                    blk_ents = {b: [] for b in blocks}
                    for s in (0, 1, 2):
                        for (b, j, el) in ents[s]:
                            blk_ents[b].append((colbase[s] + j, eloc(s, el)))
                    for b in blocks:
                        elist = blk_ents[b]
                        rows = 128 if b < NBLK - 1 else LAST_ROWS
                        ps = psopool.tile([128, D + 1], F32, space="PSUM")
                        for k, (col, el) in enumerate(elist):
                            nc.tensor.matmul(
                                ps[:, :],
                                lhsT=ohg[:, el * 128 : (el + 1) * 128],
                                rhs=T2[:, col, 0 : D + 1],
                                start=(k == 0),
                                stop=(k == len(elist) - 1),
                            )
                        recip = epool.tile([128, 1], F32, tag="recip")
                        nc.vector.reciprocal(recip[:], ps[:, D : D + 1])
                        scaled = outpool.tile([128, D], F32, tag="scaled")
                        nc.scalar.activation(
                            scaled[:], ps[:, 0:D], AF.Copy, scale=recip[:, 0:1]
                        )
                        q = outpool.tile([128, D], F32, tag="q")
                        nc.vector.tensor_tensor(q[:], scaled[:], attb_sb[:], OP.mult)
                        qs = epool.tile([128, 1], F32, tag="qs")
                        nc.vector.tensor_reduce(qs[:], q[:], AX.X, OP.add)
                        numer = epool.tile([128, 1], F32, tag="numer")
                        nc.vector.tensor_tensor(
                            numer[:], scaled[:, jmax : jmax + 1], qs[:], OP.subtract
                        )
                        nc.vector.tensor_scalar(
                            scaled[:, jmax : jmax + 1], numer[:], inv_ajmax,
                            None, OP.mult,
                        )
                        resid = outpool.tile([128, D], F32, tag="resid")
                        nc.vector.tensor_tensor(resid[:], scaled[:], xres(b), OP.add)
                        outt = outpool.tile([128, D], F32, tag="outt")
                        nc.scalar.activation(outt[:], resid[:], AF.Relu)
                        nc.sync.dma_start(
                            out_shard[b * 128 : b * 128 + rows, :], outt[:rows, :]
                        )

    nc.compile()
    return nc


def _wrap_idx(idx):
    L = len(idx)
    assert L % 16 == 0
    w = idx.reshape(L // 16, 16).T.astype(np.int16)
    return np.tile(w, (8, 1))


def _host_prep(x, edge_index, ln_gamma, ln_beta, W, att_src, att_dst, bias):
    """Fold parameters; pack edges at (group, segment) granularity."""
    Wt = W.T.astype(np.float64)
    G = ln_gamma.astype(np.float64)[:, None] * Wt
    crow = ln_beta.astype(np.float64) @ Wt
    a_src = att_src.astype(np.float64)
    v_src = G @ a_src
    v_dst = G @ att_dst.astype(np.float64)
    c_dst = float(crow @ att_dst.astype(np.float64))
    biasf = bias.astype(np.float64)

    jmax = int(np.argmax(np.abs(a_src)))
    inv_ajmax = float(1.0 / a_src[jmax])

    wext = np.zeros((D, 129), np.float32)
    wext[:, 0:D] = G.astype(np.float32)
    wext[:, jmax] = v_src.astype(np.float32)
    wext[:, 128] = v_dst.astype(np.float32)
    c2 = np.zeros((129,), np.float32)
    c2[0:D] = (crow + biasf).astype(np.float32)
    c2[jmax] = float((crow + biasf) @ a_src)
    c2[128] = c_dst - float(biasf @ a_src)
    c2b = np.broadcast_to(c2, (128, 129)).copy()

    ident = np.eye(128, dtype=np.float32)
    attb = np.broadcast_to(a_src.astype(np.float32), (128, 128)).copy()
    attb[:, jmax] = 0.0

    src = np.concatenate([edge_index[0], np.arange(N, dtype=np.int64)]).astype(np.int64)
    dst = np.concatenate([edge_index[1], np.arange(N, dtype=np.int64)]).astype(np.int64)
    core = dst // SHARD
    local = dst - core * SHARD
    blk = local // 128
    own = (src // SHARD) == core
    seg = np.where(own, 0, 1 + (src >= HALF))
    gid = blk // GBLK
    ngrp = (NBLK + GBLK - 1) // GBLK
    # sort by (core, group, seg, block)
    key = ((core * ngrp + gid) * 3 + seg) * NBLK + blk
    order = np.argsort(key, kind="stable")
    src, dst, seg, core, blk, gid = (
        src[order], dst[order], seg[order], core[order], blk[order], gid[order]
    )
    # per (core, group, seg) counts
    kgs = (core * ngrp + gid) * 3 + seg
    cnt_gs = np.bincount(kgs, minlength=NCORES * ngrp * 3).reshape(NCORES, ngrp, 3)
    ncols_gs = -(-cnt_gs.max(axis=0) // 128)               # [ngrp, 3]

    # per (core, group, seg, block) counts -> per-core block spans in cols
    kgb = ((core * ngrp + gid) * 3 + seg) * NBLK + blk
    cnt_gb = np.bincount(kgb, minlength=NCORES * ngrp * 3 * NBLK).reshape(
        NCORES, ngrp, 3, NBLK
    )

    # build layout (static, cross-core): per group/seg, per block the column
    # span [min-over-cores floor(start/128), max-over-cores ceil(end/128))
    groups = []
    col_cursor = {}
    # global fidx column numbering: all own segs (by group) first, then per
    # group lo and hi
    col0_own = {}
    cur = 0
    for gi in range(ngrp):
        col0_own[gi] = cur
        cur += int(ncols_gs[gi, 0])
    col0_lohi = {}
    for gi in range(ngrp):
        col0_lohi[(gi, 1)] = cur
        cur += int(ncols_gs[gi, 1])
        col0_lohi[(gi, 2)] = cur
        cur += int(ncols_gs[gi, 2])
    ncols_tot = cur

    # entry numbering: all own entries (by group) first, then per group lo+hi
    entries_all = {}
    ent_cursor = 0
    e0own = {}
    e0lohi = {}
    for phase in (0, 1):
        for gi in range(ngrp):
            blocks = list(range(gi * GBLK, min(NBLK, gi * GBLK + GBLK)))
            segs = (0,) if phase == 0 else (1, 2)
            if phase == 0:
                e0own[gi] = ent_cursor
            else:
                e0lohi[gi] = ent_cursor
            for s in segs:
                ncol = int(ncols_gs[gi, s])
                if ncol == 0:
                    entries_all[(gi, s)] = []
                    continue
                # per-core start offsets of each block within the seg
                starts_c = np.zeros((NCORES, len(blocks) + 1), np.int64)
                for ci in range(NCORES):
                    starts_c[ci, 1:] = np.cumsum(cnt_gb[ci, gi, s, blocks])
                ents = []
                for bi, b in enumerate(blocks):
                    lo_col = int(starts_c[:, bi].min() // 128)
                    hi_col = int(-(-starts_c[:, bi + 1].max() // 128))
                    hi_col = min(hi_col, ncol)
                    if starts_c[:, bi + 1].max() == starts_c[:, bi].min():
                        continue
                    for j in range(lo_col, hi_col):
                        ents.append((b, j, ent_cursor))
                        ent_cursor += 1
                entries_all[(gi, s)] = ents
    nent_tot = ent_cursor

    layout_groups = []
    for gi in range(ngrp):
        blocks = list(range(gi * GBLK, min(NBLK, gi * GBLK + GBLK)))
        layout_groups.append(
            {
                "blocks": blocks,
                "ncols": {s: int(ncols_gs[gi, s]) for s in range(3)},
                "col0": {0: col0_own[gi], 1: col0_lohi[(gi, 1)], 2: col0_lohi[(gi, 2)]},
                "entries": {s: entries_all[(gi, s)] for s in range(3)},
                "e0own": e0own[gi],
                "e0lohi": e0lohi[gi],
                "nent_own": len(entries_all[(gi, 0)]),
                "nent_lohi": len(entries_all[(gi, 1)]) + len(entries_all[(gi, 2)]),
            }
        )
    layout = (layout_groups, {"ncols_tot": ncols_tot, "nent_tot": nent_tot})

    # ---- per-core tables -------------------------------------------------
    feat_idx = np.zeros((NCORES, ncols_tot * 128), np.int16)
    oh = np.zeros((NCORES, 128, nent_tot * 128), NPF8)
    ohT = np.zeros((NCORES, 128, nent_tot * 128), NPF8)

    # per-core edge ranges for (core, group, seg): prefix over sorted arrays
    k_sorted = kgs  # sorted already by construction
    starts_gs = np.zeros(NCORES * ngrp * 3 + 1, np.int64)
    starts_gs[1:] = np.cumsum(cnt_gs.reshape(-1))

    for c in range(NCORES):
        for gi in range(ngrp):
            blocks = list(range(gi * GBLK, min(NBLK, gi * GBLK + GBLK)))
            for s in range(3):
                i0 = starts_gs[(c * ngrp + gi) * 3 + s]
                i1 = starts_gs[(c * ngrp + gi) * 3 + s + 1]
                n = int(i1 - i0)
                if n == 0:
                    continue
                colbase = col0_own[gi] if s == 0 else col0_lohi[(gi, s)]
                k = np.arange(n)
                base = c * SHARD if s == 0 else (0 if s == 1 else HALF)
                feat_idx[c, colbase * 128 + k] = (src[i0:i1] - base).astype(np.int16)
                # emit oh entries: edge at position k -> (col k//128, part
                # k%128), block blk[i0+k], local row dl
                dl = (dst[i0:i1] - (blk[i0:i1] * 128 + core[i0:i1] * SHARD)).astype(
                    np.int64
                )
                p = k % 128
                col = k // 128
                bb = blk[i0:i1]
                ent_of = {}
                for (b, j, el) in entries_all[(gi, s)]:
                    ent_of[(b, j)] = el
                el_arr = np.array(
                    [ent_of[(int(bb[t]), int(col[t]))] for t in range(n)],
                    dtype=np.int64,
                )
                oh[c, p, el_arr * 128 + dl] = 1.0
                ohT[c, dl, el_arr * 128 + p] = 1.0

    in_maps = []
    for c in range(NCORES):
        xs = np.zeros((PAD_SHARD, D), np.float32)
        xs[0:SHARD] = x[c * SHARD : (c + 1) * SHARD]
        in_maps.append(
            {
                "x_shard": xs,
                "wext": wext.astype(NPBF),
                "c2b": c2b,
                "ident": ident,
                "attb": attb,
                "feat_idx": _wrap_idx(feat_idx[c]),
                "oh_t": np.ascontiguousarray(oh[c]),
                "ohT_t": np.ascontiguousarray(ohT[c]),
            }
        )
    return layout, jmax, inv_ajmax, in_maps


def _layout_key(layout):
    groups, tot = layout
    parts = [tot["ncols_tot"], tot["nent_tot"]]
    for g in groups:
        parts.append(
            (
                tuple(g["blocks"]),
                tuple(sorted(g["ncols"].items())),
                tuple(sorted(g["col0"].items())),
                tuple((s, tuple(g["entries"][s])) for s in range(3)),
                g["e0own"],
                g["e0lohi"],
            )
        )
    return tuple(parts)


_PROGRAM_CACHE = {}


def kernel(x, edge_index, edge_attr, h, batch, ln_gamma, ln_beta, W, att_src,
           att_dst, bias):
    x = np.asarray(x, dtype=np.float32)
    edge_index = np.asarray(edge_index)
    h = np.asarray(h)
    ln_gamma = np.asarray(ln_gamma, dtype=np.float32)
    ln_beta = np.asarray(ln_beta, dtype=np.float32)
    W = np.asarray(W, dtype=np.float32)
    att_src = np.asarray(att_src, dtype=np.float32)
    att_dst = np.asarray(att_dst, dtype=np.float32)
    bias = np.asarray(bias, dtype=np.float32)

    layout, jmax, inv_ajmax, in_maps = _host_prep(
        x, edge_index, ln_gamma, ln_beta, W, att_src, att_dst, bias
    )
    key = (_layout_key(layout), jmax)
    if key not in _PROGRAM_CACHE:
        _PROGRAM_CACHE[key] = _build_program(layout, jmax, inv_ajmax)
    nc = _PROGRAM_CACHE[key]

    res = run_bass_kernel_spmd(nc, in_maps, core_ids=list(range(NCORES)))
    out = np.concatenate([res.results[c]["out_shard"] for c in range(NCORES)], axis=0)
    return out, h


# revision 14
# speedup vs baseline: 1.0382x; 1.0382x over previous
"""GAT layer (LayerNorm -> GATConv(heads=1) -> residual ReLU) on 8 trn2 NeuronCores.

Sharding: destination-node parallel. Each core owns N/8 nodes: it computes the
node transform for its shard, shards are AllGathered (bf16, 256 B rows), and
each core processes the edges whose destination falls in its shard.

Per-edge source records are fetched with 256 B non-transposed dma_gathers.
SWDGE descriptor generation (~7.8 ns/row, Q7-bound) is the critical resource:
  * slots are packed at (group, segment) granularity -- columns may straddle
    two dst blocks; boundary columns simply get one extra accumulating
    matmul per extra block (one-hot tables are emitted per (column, block)).
  * own-shard edges (src in the core's own shard) gather from the local
    table and are issued before the AllGather, hiding their descgen.
  * the Vector engine avoids 2-port DVE modes during phase B (they lock
    GpSimd out of SBUF): one-hot operands are host-precomputed and streamed,
    per-edge ee scaling runs on the Scalar engine.

The gathered row packs a_src INTO the feature vector: row[jmax] =
sum_f att_src[f]*xp[f] with jmax = argmax|att_src|; the aggregated feature
jmax is recovered post-scatter from the same linear identity. a_dst is looked
up on the tensor engine (ohT[r,e] one-hot times the block's a_dst column).
ee = exp(leakyrelu(a_src + a_dst)); the scatter-add is a one-hot matmul with
an ee column in the rhs producing the softmax denominator.
"""

import ml_dtypes
import numpy as np

import concourse.bacc as bacc
import concourse.mybir as mybir
import concourse.tile as tile
from concourse.bass_utils import run_bass_kernel_spmd

F32 = mybir.dt.float32
BF16 = mybir.dt.bfloat16
I16 = mybir.dt.int16
I32 = mybir.dt.int32
AX = mybir.AxisListType
OP = mybir.AluOpType
AF = mybir.ActivationFunctionType
NPBF = ml_dtypes.bfloat16
NPF8 = ml_dtypes.float8_e4m3
FP8 = mybir.dt.float8e4

N = 50000
D = 128
E = 600000
NCORES = 8
SHARD = N // NCORES            # 6250
NBLK = (SHARD + 127) // 128    # 49 dst blocks per core
PAD_SHARD = NBLK * 128         # 6272
LAST_ROWS = SHARD - (NBLK - 1) * 128  # 106
HALF = 32768
NEG_SLOPE = 0.2
LN_EPS = 1e-5
GBLK = 4                       # dst blocks per gather group
ABLK = 4                       # blocks per phase-A giga-iteration


def _build_program(layout, jmax, inv_ajmax):
    """layout: static per-group description (shared by all cores):
    list of dicts with keys
      blocks:   [b...]
      ncols:    {s: cols for seg s}            (s in 0=own,1=lo,2=hi)
      col0:     {s: global fidx column of seg s's first column}
      entries:  {s: [(b, j, eidx), ...]}       j = col within seg,
                                               eidx = global oh-entry index
      e0own / e0lohi: global entry range starts for the group's own / lo+hi
                      entry blocks (own entries contiguous; lo+hi contiguous)
      nent_own / nent_lohi
    plus layout_tot = dict(ncols_tot, nent_tot).
    """
    groups, tot = layout
    CBG_MAX = max(g["ncols"][1] + g["ncols"][2] for g in groups)
    OWN_MAX = max(g["ncols"][0] for g in groups)
    ENT_MAX = max(g["nent_own"] + g["nent_lohi"] for g in groups)
    CB = tot["ncols_tot"]
    NENT = tot["nent_tot"]

    nc = bacc.Bacc("TRN2", num_devices=NCORES, debug=False)

    x_shard = nc.dram_tensor("x_shard", [PAD_SHARD, D], F32, kind="ExternalInput")
    wext = nc.dram_tensor("wext", [D, 129], BF16, kind="ExternalInput")
    c2b = nc.dram_tensor("c2b", [128, 129], F32, kind="ExternalInput")
    ident = nc.dram_tensor("ident", [128, 128], F32, kind="ExternalInput")
    attb = nc.dram_tensor("attb", [128, 128], F32, kind="ExternalInput")
    feat_idx = nc.dram_tensor("feat_idx", [128, CB * 8], I16, kind="ExternalInput")
    oh_t = nc.dram_tensor("oh_t", [128, NENT * 128], FP8, kind="ExternalInput")
    ohT_t = nc.dram_tensor("ohT_t", [128, NENT * 128], FP8, kind="ExternalInput")
    out_shard = nc.dram_tensor("out_shard", [SHARD, D], F32, kind="ExternalOutput")

    with tile.TileContext(nc) as tc:
        with (
            tc.tile_pool(name="dram", bufs=1, space="DRAM") as dram,
            tc.tile_pool(name="consts", bufs=1) as cpool,
            tc.tile_pool(name="xres", bufs=1) as xpool,
            tc.tile_pool(name="ownt", bufs=1) as ownpool,
            tc.tile_pool(name="xpek", bufs=1) as xpekeep,
        ):
            xp_shard = dram.tile([SHARD, D], BF16)
            xp_full = dram.tile([N, D], BF16, addr_space="Shared")

            ident_sb = cpool.tile([128, 128], F32)
            nc.sync.dma_start(ident_sb[:], ident[:, :])
            attb_sb = cpool.tile([128, 128], F32)
            nc.sync.dma_start(attb_sb[:], attb[:, :])
            wext_sb = cpool.tile([D, 129], BF16)
            nc.sync.dma_start(wext_sb[:], wext[:, :])
            c2b_sb = cpool.tile([128, 129], F32)
            nc.sync.dma_start(c2b_sb[:], c2b[:, :])
            eps_sb = cpool.tile([128, 1], F32)
            nc.vector.memset(eps_sb[:], LN_EPS)
            fidx_sb = cpool.tile([128, CB * 8], I16)
            nc.sync.dma_start(fidx_sb[:], feat_idx[:, :])
            adst_sb = cpool.tile([128, NBLK], BF16)

            # x loaded 4 blocks per tile: partition p = row p of each block
            nga = (NBLK + ABLK - 1) // ABLK
            x_tiles = []
            for a in range(nga):
                nb = min(ABLK, NBLK - a * ABLK)
                xt = xpool.tile([128, ABLK, D], F32, tag=f"x4_{a}")
                nc.sync.dma_start(
                    xt[:, 0:nb, :],
                    x_shard[a * ABLK * 128 : (a * ABLK + nb) * 128, :].rearrange(
                        "(a p) b -> p a b", p=128
                    ),
                )
                x_tiles.append(xt)

            def xres(b):
                return x_tiles[b // ABLK][:, b % ABLK, :]

            xpe_tiles = []
            # ---------------- Phase A: node transform on own shard ---------
            with (
                tc.tile_pool(name="a_small", bufs=6) as spool,
                tc.tile_pool(name="a_sq", bufs=3) as sqpool,
                tc.tile_pool(name="a_xnp", bufs=6) as xnppool,
                tc.tile_pool(name="a_xnpT", bufs=6) as xnptpool,
                tc.tile_pool(name="a_ps_t", bufs=3, space="PSUM") as psa,
                tc.tile_pool(name="a_ps_m", bufs=3, space="PSUM") as psb,
            ):
                for a in range(nga):
                    nb = min(ABLK, NBLK - a * ABLK)
                    xt = x_tiles[a]
                    sum4 = spool.tile([128, ABLK], F32, tag="sum4")
                    nc.vector.tensor_reduce(sum4[:, 0:nb], xt[:, 0:nb, :], AX.X, OP.add)
                    sq4 = sqpool.tile([128, ABLK, D], F32)
                    nc.scalar.activation(sq4[:, 0:nb, :], xt[:, 0:nb, :], AF.Square)
                    ssq4 = spool.tile([128, ABLK], F32, tag="ssq4")
                    nc.vector.tensor_reduce(ssq4[:, 0:nb], sq4[:, 0:nb, :], AX.X, OP.add)
                    mu4 = spool.tile([128, ABLK], F32, tag="mu4")
                    nc.vector.tensor_scalar(mu4[:, 0:nb], sum4[:, 0:nb], 1.0 / D, None, OP.mult)
                    m24 = spool.tile([128, ABLK], F32, tag="m24")
                    nc.vector.tensor_tensor(m24[:, 0:nb], mu4[:, 0:nb], mu4[:, 0:nb], OP.mult)
                    var4 = spool.tile([128, ABLK], F32, tag="var4")
                    nc.vector.tensor_scalar(
                        var4[:, 0:nb], ssq4[:, 0:nb], 1.0 / D, None, OP.mult
                    )
                    nc.vector.tensor_tensor(
                        var4[:, 0:nb], var4[:, 0:nb], m24[:, 0:nb], OP.subtract
                    )
                    std4 = spool.tile([128, ABLK], F32, tag="std4")
                    nc.scalar.activation(std4[:, 0:nb], var4[:, 0:nb], AF.Sqrt, bias=eps_sb[:, 0:1])
                    rstd4 = spool.tile([128, ABLK], F32, tag="rstd4")
                    nc.vector.reciprocal(rstd4[:, 0:nb], std4[:, 0:nb])
                    for k in range(nb):
                        i = a * ABLK + k
                        rows = 128 if i < NBLK - 1 else LAST_ROWS
                        xnp = xnppool.tile([128, D], F32)
                        nc.vector.tensor_scalar(
                            xnp[:], xt[:, k, :], mu4[:, k : k + 1],
                            rstd4[:, k : k + 1], OP.subtract, OP.mult,
                        )
                        pt = psa.tile([128, 128], F32, space="PSUM")
                        nc.tensor.transpose(pt[:], xnp[:], ident_sb[:])
                        xnpT = xnptpool.tile([128, 128], BF16)
                        nc.scalar.copy(xnpT[:], pt[:])
                        pm = psb.tile([128, 129], F32, space="PSUM")
                        nc.tensor.matmul(
                            pm[:], lhsT=xnpT[:], rhs=wext_sb[:], start=True, stop=True
                        )
                        xpe = xpekeep.tile([128, 129], BF16, tag=f"xpe{i}")
                        xpe_tiles.append(xpe)
                        nc.vector.tensor_tensor(xpe[:], pm[:], c2b_sb[:], OP.add)
                        nc.sync.dma_start(
                            xp_shard[i * 128 : i * 128 + rows, :], xpe[:rows, 0:128]
                        )
                        nc.vector.tensor_copy(adst_sb[:, i : i + 1], xpe[:, 128:129])

            # AllGather triggers first (runs on the CC cores), then the
            # own-shard gathers' descgen overlaps the collective transfer
            nc.gpsimd.collective_compute(
                "AllGather",
                OP.bypass,
                replica_groups=[list(range(NCORES))],
                ins=[xp_shard[:, :]],
                outs=[xp_full[:, :]],
            )
            ownT = []
            for gi, g in enumerate(groups):
                ncol = g["ncols"][0]
                t = ownpool.tile([128, max(ncol, 1), D], BF16, tag=f"ownT{gi}")
                if ncol:
                    c0 = g["col0"][0]
                    nc.gpsimd.dma_gather(
                        out_ap=t[:, 0:ncol, :],
                        in_ap=xp_shard[:, :],
                        idxs_ap=fidx_sb[:, c0 * 8 : (c0 + ncol) * 8],
                        num_idxs=ncol * 128,
                        num_idxs_reg=ncol * 128,
                        elem_size=D,
                        single_packet=False,
                    )
                ownT.append(t)

            # ---------------- Phase B: edge aggregation --------------------
            with (
                tc.tile_pool(name="b_g", bufs=3) as gpool,
                tc.tile_pool(name="b_oh", bufs=2) as opool,
                tc.tile_pool(name="b_ohT", bufs=2) as otpool,
                tc.tile_pool(name="b_f", bufs=2) as fpool,
                tc.tile_pool(name="b_e", bufs=3) as epool,
                tc.tile_pool(name="b_o", bufs=4) as outpool,
                tc.tile_pool(name="b_pso", bufs=4, space="PSUM") as psopool,
                tc.tile_pool(name="b_psa", bufs=2, space="PSUM") as psapool,
            ):
                for gi, g in enumerate(groups):
                    blocks = g["blocks"]
                    nown = g["ncols"][0]
                    nlo, nhi = g["ncols"][1], g["ncols"][2]
                    cbl = nlo + nhi          # lo+hi cols in T
                    cbg = nown + cbl         # total cols this group
                    T = gpool.tile([128, CBG_MAX, D], BF16, tag="T")
                    if nlo:
                        c0 = g["col0"][1]
                        nc.gpsimd.dma_gather(
                            out_ap=T[:, 0:nlo, :],
                            in_ap=xp_full[0:HALF, :],
                            idxs_ap=fidx_sb[:, c0 * 8 : (c0 + nlo) * 8],
                            num_idxs=nlo * 128,
                            num_idxs_reg=nlo * 128,
                            elem_size=D,
                            single_packet=False,
                        )
                    if nhi:
                        c0 = g["col0"][2]
                        nc.gpsimd.dma_gather(
                            out_ap=T[:, nlo : nlo + nhi, :],
                            in_ap=xp_full[HALF:N, :],
                            idxs_ap=fidx_sb[:, c0 * 8 : (c0 + nhi) * 8],
                            num_idxs=nhi * 128,
                            num_idxs_reg=nhi * 128,
                            elem_size=D,
                            single_packet=False,
                        )

                    # oh/ohT entries: [own entries | lo+hi entries]
                    ne_own, ne_lohi = g["nent_own"], g["nent_lohi"]
                    nent = ne_own + ne_lohi
                    ohg = opool.tile([128, ENT_MAX * 128], FP8, tag="ohg")
                    ohTg = otpool.tile([128, ENT_MAX * 128], FP8, tag="ohTg")
                    if ne_own:
                        e0 = g["e0own"]
                        nc.sync.dma_start(
                            ohg[:, 0 : ne_own * 128],
                            oh_t[:, e0 * 128 : (e0 + ne_own) * 128],
                        )
                        nc.sync.dma_start(
                            ohTg[:, 0 : ne_own * 128],
                            ohT_t[:, e0 * 128 : (e0 + ne_own) * 128],
                        )
                    e0 = g["e0lohi"]
                    nc.sync.dma_start(
                        ohg[:, ne_own * 128 : nent * 128],
                        oh_t[:, e0 * 128 : (e0 + ne_lohi) * 128],
                    )
                    nc.sync.dma_start(
                        ohTg[:, ne_own * 128 : nent * 128],
                        ohT_t[:, e0 * 128 : (e0 + ne_lohi) * 128],
                    )

                    def tcol(j):
                        """(tile, local col) for group column j (own first)."""
                        if j < nown:
                            return ownT[gi], j
                        return T, j - nown

                    # gate: a_dst columns data-dependent on the gathers so the
                    # a_dst matmuls can't hoist into phase A
                    gn = len(blocks)
                    g0 = blocks[0]
                    tz = epool.tile([128, 1], F32, tag="tz")
                    src_gate = T if cbl else ownT[gi]
                    nc.vector.tensor_scalar(tz[:], src_gate[:, 0, 0:1], 0.0, None, OP.mult)
                    adst_g = epool.tile([128, GBLK], BF16, tag="adst_g")
                    nc.vector.tensor_scalar(
                        adst_g[:, 0:gn], adst_sb[:, g0 : g0 + gn], tz[:, 0:1],
                        None, OP.add,
                    )

                    # a_dst lookups: per (column, block) entry, accumulated
                    # into the column's ps_adst slot
                    ps_adst = psapool.tile([128, max(cbg, 1)], F32, space="PSUM")
                    ents = g["entries"]  # {s: [(b, j_seg, eidx_global)]}
                    # group-local column index: own seg cols [0, nown);
                    # lo cols [nown, nown+nlo); hi cols [nown+nlo, cbg)
                    colbase = {0: 0, 1: nown, 2: nown + nlo}

                    def eloc(s, el):
                        """global oh entry index -> column in ohg/ohTg tile"""
                        if s == 0:
                            return el - g["e0own"]
                        return el - g["e0lohi"] + ne_own

                    # first/last entry per column for start/stop flags
                    col_ents = {}
                    for s in (0, 1, 2):
                        for (b, j, el) in ents[s]:
                            col_ents.setdefault(colbase[s] + j, []).append(
                                (b, eloc(s, el))
                            )
                    for col in sorted(col_ents):
                        for idx, (b, el) in enumerate(col_ents[col]):
                            nc.tensor.matmul(
                                ps_adst[:, col : col + 1],
                                lhsT=ohTg[:, el * 128 : (el + 1) * 128],
                                rhs=adst_g[:, b - g0 : b - g0 + 1],
                                start=(idx == 0),
                                stop=(idx == len(col_ents[col]) - 1),
                            )

                    # ee = exp(leakyrelu(a_src + a_dst)) for the whole group
                    adst_bg = epool.tile([128, max(cbg, 1)], BF16, tag="adst_bg")
                    nc.vector.tensor_copy(adst_bg[:, 0:cbg], ps_adst[:, 0:cbg])
                    e1 = epool.tile([128, max(cbg, 1)], BF16, tag="e1")
                    if nown:
                        nc.vector.tensor_tensor(
                            e1[:, 0:nown], ownT[gi][:, 0:nown, jmax],
                            adst_bg[:, 0:nown], OP.add,
                        )
                    if cbl:
                        nc.vector.tensor_tensor(
                            e1[:, nown:cbg], T[:, 0:cbl, jmax],
                            adst_bg[:, nown:cbg], OP.add,
                        )
                    e2 = epool.tile([128, max(cbg, 1)], BF16, tag="e2")
                    nc.vector.tensor_scalar(
                        e2[:, 0:cbg], e1[:, 0:cbg], NEG_SLOPE, None, OP.mult
                    )
                    e3 = epool.tile([128, max(cbg, 1)], BF16, tag="e3")
                    nc.vector.tensor_tensor(
                        e3[:, 0:cbg], e2[:, 0:cbg], e1[:, 0:cbg], OP.max
                    )
                    ee = epool.tile([128, max(cbg, 1)], F32, tag="ee")
                    nc.scalar.activation(ee[:, 0:cbg], e3[:, 0:cbg], AF.Exp)

                    # rhs rows scaled by ee (Scalar engine), ee denom column
                    T2 = fpool.tile([128, CBG_MAX + OWN_MAX, D + 1], BF16, tag="T2")
                    for col in range(cbg):
                        tt, lj = tcol(col)
                        if col % 2 == 0:
                            nc.scalar.activation(
                                T2[:, col, 0:D], tt[:, lj, :], AF.Copy,
                                scale=ee[:, col : col + 1],
                            )
                        else:
                            nc.vector.tensor_scalar(
                                T2[:, col, 0:D], tt[:, lj, :],
                                ee[:, col : col + 1], None, OP.mult,
                            )
                    nc.vector.tensor_copy(T2[:, 0:cbg, D], ee[:, 0:cbg])

                    # per-block scatter: all entries of block b accumulate
                    blk_ents = {b: [] for b in blocks}
                    for s in (0, 1, 2):
                        for (b, j, el) in ents[s]:
                            blk_ents[b].append((colbase[s] + j, eloc(s, el)))
                    for b in blocks:
                        elist = blk_ents[b]
                        rows = 128 if b < NBLK - 1 else LAST_ROWS
                        ps = psopool.tile([128, D + 1], F32, space="PSUM")
                        for k, (col, el) in enumerate(elist):
                            nc.tensor.matmul(
                                ps[:, :],
                                lhsT=ohg[:, el * 128 : (el + 1) * 128],
                                rhs=T2[:, col, 0 : D + 1],
                                start=(k == 0),
                                stop=(k == len(elist) - 1),
                            )
                        # self-loop: ee_self from the block's own xpe tile
                        xb = xpe_tiles[b]
                        e1s = epool.tile([128, 1], BF16, tag="e1s")
                        nc.vector.tensor_tensor(
                            e1s[:], xb[:, jmax : jmax + 1], xb[:, 128:129], OP.add
                        )
                        e2s = epool.tile([128, 1], BF16, tag="e2s")
                        nc.vector.tensor_scalar(
                            e2s[:], e1s[:], NEG_SLOPE, None, OP.mult
                        )
                        e3s = epool.tile([128, 1], BF16, tag="e3s")
                        nc.vector.tensor_tensor(e3s[:], e2s[:], e1s[:], OP.max)
                        ees = epool.tile([128, 1], F32, tag="ees")
                        nc.scalar.activation(ees[:], e3s[:], AF.Exp)
                        contrib = outpool.tile([128, D], F32, tag="contrib")
                        nc.vector.tensor_scalar(
                            contrib[:], xb[:, 0:D], ees[:, 0:1], None, OP.mult
                        )
                        denom2 = epool.tile([128, 1], F32, tag="denom2")
                        nc.vector.tensor_tensor(
                            denom2[:], ps[:, D : D + 1], ees[:], OP.add
                        )
                        num2 = outpool.tile([128, D], F32, tag="num2")
                        nc.vector.tensor_tensor(
                            num2[:], ps[:, 0:D], contrib[:], OP.add
                        )
                        recip = epool.tile([128, 1], F32, tag="recip")
                        nc.vector.reciprocal(recip[:], denom2[:])
                        scaled = outpool.tile([128, D], F32, tag="scaled")
                        nc.scalar.activation(
                            scaled[:], num2[:], AF.Copy, scale=recip[:, 0:1]
                        )
                        q = outpool.tile([128, D], F32, tag="q")
                        nc.vector.tensor_tensor(q[:], scaled[:], attb_sb[:], OP.mult)
                        qs = epool.tile([128, 1], F32, tag="qs")
                        nc.vector.tensor_reduce(qs[:], q[:], AX.X, OP.add)
                        numer = epool.tile([128, 1], F32, tag="numer")
                        nc.vector.tensor_tensor(
                            numer[:], scaled[:, jmax : jmax + 1], qs[:], OP.subtract
                        )
                        nc.vector.tensor_scalar(
                            scaled[:, jmax : jmax + 1], numer[:], inv_ajmax,
                            None, OP.mult,
                        )
                        resid = outpool.tile([128, D], F32, tag="resid")
                        nc.vector.tensor_tensor(resid[:], scaled[:], xres(b), OP.add)
                        outt = outpool.tile([128, D], F32, tag="outt")
                        nc.scalar.activation(outt[:], resid[:], AF.Relu)
                        nc.sync.dma_start(
                            out_shard[b * 128 : b * 128 + rows, :], outt[:rows, :]
                        )

    nc.compile()
    return nc


def _wrap_idx(idx):
    L = len(idx)
    assert L % 16 == 0
    w = idx.reshape(L // 16, 16).T.astype(np.int16)
    return np.tile(w, (8, 1))


def _host_prep(x, edge_index, ln_gamma, ln_beta, W, att_src, att_dst, bias):
    """Fold parameters; pack edges at (group, segment) granularity."""
    Wt = W.T.astype(np.float64)
    G = ln_gamma.astype(np.float64)[:, None] * Wt
    crow = ln_beta.astype(np.float64) @ Wt
    a_src = att_src.astype(np.float64)
    v_src = G @ a_src
    v_dst = G @ att_dst.astype(np.float64)
    c_dst = float(crow @ att_dst.astype(np.float64))
    biasf = bias.astype(np.float64)

    jmax = int(np.argmax(np.abs(a_src)))
    inv_ajmax = float(1.0 / a_src[jmax])

    wext = np.zeros((D, 129), np.float32)
    wext[:, 0:D] = G.astype(np.float32)
    wext[:, jmax] = v_src.astype(np.float32)
    wext[:, 128] = v_dst.astype(np.float32)
    c2 = np.zeros((129,), np.float32)
    c2[0:D] = (crow + biasf).astype(np.float32)
    c2[jmax] = float((crow + biasf) @ a_src)
    c2[128] = c_dst - float(biasf @ a_src)
    c2b = np.broadcast_to(c2, (128, 129)).copy()

    ident = np.eye(128, dtype=np.float32)
    attb = np.broadcast_to(a_src.astype(np.float32), (128, 128)).copy()
    attb[:, jmax] = 0.0

    # self loops are handled on-chip from the phase-A tiles, not gathered
    src = edge_index[0].astype(np.int64)
    dst = edge_index[1].astype(np.int64)
    core = dst // SHARD
    local = dst - core * SHARD
    blk = local // 128
    own = (src // SHARD) == core
    seg = np.where(own, 0, 1 + (src >= HALF))
    gid = blk // GBLK
    ngrp = (NBLK + GBLK - 1) // GBLK
    # sort by (core, group, seg, block)
    key = ((core * ngrp + gid) * 3 + seg) * NBLK + blk
    order = np.argsort(key, kind="stable")
    src, dst, seg, core, blk, gid = (
        src[order], dst[order], seg[order], core[order], blk[order], gid[order]
    )
    # per (core, group, seg) counts
    kgs = (core * ngrp + gid) * 3 + seg
    cnt_gs = np.bincount(kgs, minlength=NCORES * ngrp * 3).reshape(NCORES, ngrp, 3)
    ncols_gs = -(-cnt_gs.max(axis=0) // 128)               # [ngrp, 3]

    # per (core, group, seg, block) counts -> per-core block spans in cols
    kgb = ((core * ngrp + gid) * 3 + seg) * NBLK + blk
    cnt_gb = np.bincount(kgb, minlength=NCORES * ngrp * 3 * NBLK).reshape(
        NCORES, ngrp, 3, NBLK
    )

    # build layout (static, cross-core): per group/seg, per block the column
    # span [min-over-cores floor(start/128), max-over-cores ceil(end/128))
    groups = []
    col_cursor = {}
    # global fidx column numbering: all own segs (by group) first, then per
    # group lo and hi
    col0_own = {}
    cur = 0
    for gi in range(ngrp):
        col0_own[gi] = cur
        cur += int(ncols_gs[gi, 0])
    col0_lohi = {}
    for gi in range(ngrp):
        col0_lohi[(gi, 1)] = cur
        cur += int(ncols_gs[gi, 1])
        col0_lohi[(gi, 2)] = cur
        cur += int(ncols_gs[gi, 2])
    ncols_tot = cur

    # entry numbering: all own entries (by group) first, then per group lo+hi
    entries_all = {}
    ent_cursor = 0
    e0own = {}
    e0lohi = {}
    for phase in (0, 1):
        for gi in range(ngrp):
            blocks = list(range(gi * GBLK, min(NBLK, gi * GBLK + GBLK)))
            segs = (0,) if phase == 0 else (1, 2)
            if phase == 0:
                e0own[gi] = ent_cursor
            else:
                e0lohi[gi] = ent_cursor
            for s in segs:
                ncol = int(ncols_gs[gi, s])
                if ncol == 0:
                    entries_all[(gi, s)] = []
                    continue
                # per-core start offsets of each block within the seg
                starts_c = np.zeros((NCORES, len(blocks) + 1), np.int64)
                for ci in range(NCORES):
                    starts_c[ci, 1:] = np.cumsum(cnt_gb[ci, gi, s, blocks])
                ents = []
                for bi, b in enumerate(blocks):
                    lo_col = int(starts_c[:, bi].min() // 128)
                    hi_col = int(-(-starts_c[:, bi + 1].max() // 128))
                    hi_col = min(hi_col, ncol)
                    if starts_c[:, bi + 1].max() == starts_c[:, bi].min():
                        continue
                    for j in range(lo_col, hi_col):
                        ents.append((b, j, ent_cursor))
                        ent_cursor += 1
                entries_all[(gi, s)] = ents
    nent_tot = ent_cursor

    layout_groups = []
    for gi in range(ngrp):
        blocks = list(range(gi * GBLK, min(NBLK, gi * GBLK + GBLK)))
        layout_groups.append(
            {
                "blocks": blocks,
                "ncols": {s: int(ncols_gs[gi, s]) for s in range(3)},
                "col0": {0: col0_own[gi], 1: col0_lohi[(gi, 1)], 2: col0_lohi[(gi, 2)]},
                "entries": {s: entries_all[(gi, s)] for s in range(3)},
                "e0own": e0own[gi],
                "e0lohi": e0lohi[gi],
                "nent_own": len(entries_all[(gi, 0)]),
                "nent_lohi": len(entries_all[(gi, 1)]) + len(entries_all[(gi, 2)]),
            }
        )
    layout = (layout_groups, {"ncols_tot": ncols_tot, "nent_tot": nent_tot})

    # ---- per-core tables -------------------------------------------------
    feat_idx = np.zeros((NCORES, ncols_tot * 128), np.int16)
    oh = np.zeros((NCORES, 128, nent_tot * 128), NPF8)
    ohT = np.zeros((NCORES, 128, nent_tot * 128), NPF8)

    # per-core edge ranges for (core, group, seg): prefix over sorted arrays
    k_sorted = kgs  # sorted already by construction
    starts_gs = np.zeros(NCORES * ngrp * 3 + 1, np.int64)
    starts_gs[1:] = np.cumsum(cnt_gs.reshape(-1))

    for c in range(NCORES):
        for gi in range(ngrp):
            blocks = list(range(gi * GBLK, min(NBLK, gi * GBLK + GBLK)))
            for s in range(3):
                i0 = starts_gs[(c * ngrp + gi) * 3 + s]
                i1 = starts_gs[(c * ngrp + gi) * 3 + s + 1]
                n = int(i1 - i0)
                if n == 0:
                    continue
                colbase = col0_own[gi] if s == 0 else col0_lohi[(gi, s)]
                k = np.arange(n)
                base = c * SHARD if s == 0 else (0 if s == 1 else HALF)
                feat_idx[c, colbase * 128 + k] = (src[i0:i1] - base).astype(np.int16)
                # emit oh entries: edge at position k -> (col k//128, part
                # k%128), block blk[i0+k], local row dl
                dl = (dst[i0:i1] - (blk[i0:i1] * 128 + core[i0:i1] * SHARD)).astype(
                    np.int64
                )
                p = k % 128
                col = k // 128
                bb = blk[i0:i1]
                ent_of = {}
                for (b, j, el) in entries_all[(gi, s)]:
                    ent_of[(b, j)] = el
                el_arr = np.array(
                    [ent_of[(int(bb[t]), int(col[t]))] for t in range(n)],
                    dtype=np.int64,
                )
                oh[c, p, el_arr * 128 + dl] = 1.0
                ohT[c, dl, el_arr * 128 + p] = 1.0

    in_maps = []
    for c in range(NCORES):
        xs = np.zeros((PAD_SHARD, D), np.float32)
        xs[0:SHARD] = x[c * SHARD : (c + 1) * SHARD]
        in_maps.append(
            {
                "x_shard": xs,
                "wext": wext.astype(NPBF),
                "c2b": c2b,
                "ident": ident,
                "attb": attb,
                "feat_idx": _wrap_idx(feat_idx[c]),
                "oh_t": np.ascontiguousarray(oh[c]),
                "ohT_t": np.ascontiguousarray(ohT[c]),
            }
        )
    return layout, jmax, inv_ajmax, in_maps


def _layout_key(layout):
    groups, tot = layout
    parts = [tot["ncols_tot"], tot["nent_tot"]]
    for g in groups:
        parts.append(
            (
                tuple(g["blocks"]),
                tuple(sorted(g["ncols"].items())),
                tuple(sorted(g["col0"].items())),
                tuple((s, tuple(g["entries"][s])) for s in range(3)),
                g["e0own"],
                g["e0lohi"],
            )
        )
    return tuple(parts)


_PROGRAM_CACHE = {}


def kernel(x, edge_index, edge_attr, h, batch, ln_gamma, ln_beta, W, att_src,
           att_dst, bias):
    x = np.asarray(x, dtype=np.float32)
    edge_index = np.asarray(edge_index)
    h = np.asarray(h)
    ln_gamma = np.asarray(ln_gamma, dtype=np.float32)
    ln_beta = np.asarray(ln_beta, dtype=np.float32)
    W = np.asarray(W, dtype=np.float32)
    att_src = np.asarray(att_src, dtype=np.float32)
    att_dst = np.asarray(att_dst, dtype=np.float32)
    bias = np.asarray(bias, dtype=np.float32)

    layout, jmax, inv_ajmax, in_maps = _host_prep(
        x, edge_index, ln_gamma, ln_beta, W, att_src, att_dst, bias
    )
    key = (_layout_key(layout), jmax)
    if key not in _PROGRAM_CACHE:
        _PROGRAM_CACHE[key] = _build_program(layout, jmax, inv_ajmax)
    nc = _PROGRAM_CACHE[key]

    res = run_bass_kernel_spmd(nc, in_maps, core_ids=list(range(NCORES)))
    out = np.concatenate([res.results[c]["out_shard"] for c in range(NCORES)], axis=0)
    return out, h
